# revision 20
# baseline (speedup 1.0000x reference)
"""Chamfer loss kernel for Trainium2 (8 NeuronCores, data-parallel over batch).

Math: for each batch b,
  d2[n, m] = ||x_n||^2 + ||y_m||^2 - 2 x_n . y_m
  mean1 = mean over (b, n) of min_m d2,  mean2 = mean over (b, m) of min_n d2
  out = max(mean1, mean2)

Device strategy (per core, 2 batches): one K=9 matmul per [128 x 512] tile
computes c = x.y - ||x||^2/2 - ||y||^2/2 = -d2/2 directly in PSUM, using
host-prestacked operands [x^2, x, -0.5] against [-0.5, y, y^2]. Matmuls run
as float32r (1 cycle/row vs 4 for plain fp32). The min-reduction is split
across two engines: for each pair of [128 x 1024] PSUM tiles, ACT copies one
tile to SBUF while DVE consumes the other directly, then a single
tensor_tensor_reduce (fused elementwise max + free-axis max) scans both
streams at one column/cycle — PSUM read port and SBUF read port in parallel
(min d2 == -2 max c). Both Chamfer directions run as separate passes with
the stacked operands swapped. Partials are merged on-chip to two sums; the
host scales and takes the max.
"""

import sys

if "/opt/trn_rl_repo" not in sys.path:
    sys.path.insert(0, "/opt/trn_rl_repo")

import numpy as np

B = 16  # total batches
NPTS = 4096  # points per cloud (N == M)
NCORES = 8
BPC = B // NCORES  # batches per core
SUB = 1024  # PSUM sub-tile free size (2 banks)
KR = 21  # stacked contraction rows (hi/lo-split operands, see _make_in_maps)
NPAIR = NPTS // (2 * SUB)  # TTR pairs per row-tile
NI = NPTS // 128  # n-tiles per batch

_STATE: dict = {}


def _build(strip: bool = True, ni: int = NI, reps: int = 1, op_dt: str = "f32r"):
    import concourse.bass as bass
    import concourse.mybir as mybir
    from concourse import tile

    f32 = mybir.dt.float32
    mm_dt = mybir.dt.float32r if op_dt == "f32r" else mybir.dt.float32
    Alu = mybir.AluOpType

    nc = bass.Bass("TRN2", target_bir_lowering=False, debug=False)
    # Row stacks are built on the host with every operand split into an
    # 11-bit-mantissa hi part (exactly representable in fp32r) and its
    # residual lo part, so the K=21 contraction reconstructs near-f32
    # precision on fp32r hardware (see _make_in_maps for the layout).
    xin = nc.dram_tensor("xin", [BPC, KR, NPTS], mm_dt, kind="ExternalInput")
    yin = nc.dram_tensor("yin", [BPC, KR, NPTS], mm_dt, kind="ExternalInput")
    out = nc.dram_tensor("out", [1, 2], f32, kind="ExternalOutput")

    NCOL = NPAIR * ni * BPC  # per-direction partial count

    with tile.TileContext(nc) as tc:
        with (
            tc.tile_pool(name="singles", bufs=1) as singles,
            tc.tile_pool(name="copies", bufs=4) as copies,
            tc.tile_pool(name="psum", bufs=4, space="PSUM") as psum_pool,
        ):
            # batch blocks at partitions 0/32 (compute engines address
            # bases 0/32/64/96).
            xs = singles.tile([32 * (BPC - 1) + KR, NPTS], mm_dt)
            ys = singles.tile([32 * (BPC - 1) + KR, NPTS], mm_dt)
            rowbuf = singles.tile([128, 2 * NPAIR * NI * BPC], f32)
            rowmins = singles.tile([128, 2 * NI * BPC], f32)
            ones = singles.tile([128, 1], f32)
            totals = singles.tile([128, 2], f32)
            out_sb = singles.tile([1, 2], f32)

            nc.vector.memset(ones[:], 1.0)

            def absorber_ap(pt, ap):
                # Tiny matmul that makes PE observe one producer of `ap`
                # (walrus fits one sync wait per Matmult; spreading producer
                # waits over this chain keeps every later matmul legal). Its
                # 1-elem output is overwritten by the next start=True matmul.
                # Plain f32 view: fp32r matmuls have ISA restrictions that
                # reject 1-elem outputs.
                ap32 = ap.bitcast(f32)
                nc.tensor.matmul(
                    pt[0:1, 0:1],
                    ap32,
                    ap32,
                    start=True,
                    stop=True,
                    skip_group_check=True,
                )

            for b in range(BPC):
                r = 32 * b
                nc.sync.dma_start(xs[r : r + KR, :], xin[b])
                nc.sync.dma_start(ys[r : r + KR, :], yin[b])

            for rep in range(reps):
                for p, (lstk, rstk) in enumerate([(xs, ys), (ys, xs)]):
                    for b in range(BPC):
                        r = 32 * b
                        for i in range(ni):
                            lhs = lstk[r : r + KR, i * 128 : (i + 1) * 128]
                            for j in range(NPAIR):
                                t_ps = psum_pool.tile([128, SUB], f32, tag="mm")
                                t_cp = psum_pool.tile([128, SUB], f32, tag="mm")
                                if rep == 0 and p == 0 and i == 0 and j == 0:
                                    # one absorber per input DMA
                                    absorber_ap(t_ps, xs[r : r + KR, 0:1])
                                    absorber_ap(t_ps, ys[r : r + KR, 0:1])
                                    if b + 1 < BPC:
                                        r2 = 32 * (b + 1)
                                        absorber_ap(t_ps, xs[r2 : r2 + KR, 0:1])
                                        absorber_ap(t_ps, ys[r2 : r2 + KR, 0:1])
                                # Fill t_ps first, t_cp second: ACT's copy of
                                # t_cp then transitively implies t_ps is done
                                # (PE completes in order), so the scan's PE
                                # wait and the copy's DVE wait can be DROPPED
                                # in _legalize_waits — every instruction fits
                                # walrus's one-sync-wait budget with no NoOp
                                # carriers (which hardware does not honor).
                                for h, pt in ((0, t_ps), (1, t_cp)):
                                    for q in range(SUB // 512):
                                        c0 = j * 2 * SUB + h * SUB + q * 512
                                        nc.tensor.matmul(
                                            pt[:, q * 512 : (q + 1) * 512],
                                            lhs,
                                            rstk[r : r + KR, c0 : c0 + 512],
                                            start=True,
                                            stop=True,
                                        )
                                sb = copies.tile([128, SUB], f32, tag="cp")
                                nc.scalar.copy(sb[:], t_cp[:])
                                # Running max over both streams:
                                #   state = max(data0_t, state, data1_t)
                                # The stride-0 broadcast out leaves the final
                                # state — the row max over this 2*SUB chunk —
                                # in rowbuf[:, col]. Each pair gets its OWN
                                # column: chaining pair j=1 off pair j=0's
                                # column (initial=col) reads the previous
                                # scan's final write before it retires from
                                # the DVE store pipeline and picks up stale
                                # SBUF data.
                                col = (p * NCOL + (b * ni + i) * NPAIR) + j
                                colap = rowbuf[:, col : col + 1]
                                nc.vector.tensor_tensor_scan(
                                    out=colap.broadcast_to(t_ps.shape),
                                    data0=t_ps[:],
                                    data1=sb[:],
                                    initial=-1e30,
                                    op0=Alu.max,
                                    op1=Alu.max,
                                )

                # Merge per-pair partials: max over the NPAIR partials of
                # each row-tile, then sum per direction. Values are
                # -min(d2)/2; the host applies the -2 scale.
                nc.vector.tensor_reduce(
                    out=rowmins[:, 0 : 2 * ni * BPC],
                    in_=rowbuf[:, 0 : 2 * NCOL].rearrange(
                        "p (i s) -> p i s", s=NPAIR
                    ),
                    axis=mybir.AxisListType.X,
                    op=Alu.max,
                )
                h = ni * BPC
                nc.vector.tensor_reduce(
                    out=totals[:, 0:1],
                    in_=rowmins[:, 0:h],
                    axis=mybir.AxisListType.X,
                    op=Alu.add,
                )
                nc.vector.tensor_reduce(
                    out=totals[:, 1:2],
                    in_=rowmins[:, h : 2 * h],
                    axis=mybir.AxisListType.X,
                    op=Alu.add,
                )
                outp = psum_pool.tile([128, SUB], f32, tag="mm")
                nc.tensor.matmul(
                    outp[0:1, 0:2], ones[:], totals[:], start=True, stop=True
                )
                nc.scalar.copy(out_sb[:], outp[0:1, 0:2])
                nc.sync.dma_start(out[:], out_sb[:])

    if strip:
        _legalize_waits(nc)
    return nc


_ENGINE_SEM_PREFIX = {
    "EngineType.PE": "PE_",
    "EngineType.DVE": "DVE_",
    "EngineType.Activation": "Activation_",
    "EngineType.Pool": "Pool_",
}

# Waits that may be dropped when an instruction exceeds walrus's one-wait
# budget, in drop order. Each entry is provably implied by a wait that stays
# (or is a benign DMA-queue credit):
#  - DVE (the scans) drops PE_: the kept Activation_ wait is on the copy of
#    t_cp, which itself waited for PE past the t_ps fill (PE runs in order).
#  - Activation (the copies) drops DVE_: the kept PE_ wait is on t_cp, which
#    follows t_ps in PE order, and t_ps's fill waited on the DVE scan that
#    last read the reused SBUF copy buffer. DMAHW credits only gate the
#    previous rep's already-superseded out DMA.
#  - PE (the tail matmul) drops Activation_: the kept DVE_ wait is on totals,
#    downstream of every scan, each of which waited its ACT copy.
#  - SP (out DMA) drops DMAHW queue credits (2 descriptors per rep, depth 8).
_DROPPABLE = {
    "EngineType.DVE": ("PE_",),
    "EngineType.Activation": ("DVE_", "DMAHW"),
    "EngineType.PE": ("Activation_",),
    "EngineType.SP": ("DMAHW",),
}

_WAIT_LIMITS: dict = {}
_DEFAULT_WAIT_LIMIT = 1


def _legalize_waits(nc):
    """Make every instruction's sync-wait count fit its walrus struct
    (one wait per instruction on this stack).

    First drop waits on the instruction's own engine (each engine executes
    its stream in order, so same-engine waits are redundant on hardware),
    then waits listed in _DROPPABLE for the engine (transitively implied by
    the kept wait — see the table). Any still-excess waits move onto NoOp
    carrier instructions prepended on the same engine queue. NOTE: hardware
    does not reliably honor NoOp carriers, so the kernel structure must keep
    every needed ordering expressible in one wait; the carrier path is a
    last resort for cold-path instructions (the final Drain).
    """
    import concourse.mybir as mybir

    carrier_id = [0]
    for fn in nc.m.functions:
        for blk in fn.blocks:
            insts = blk.instructions
            out = []
            changed = False
            for inst in insts:
                si = inst.sync_info
                limit = _WAIT_LIMITS.get(str(inst.opcode), _DEFAULT_WAIT_LIMIT)
                if si is None or not si.on_wait or len(si.on_wait) <= limit:
                    out.append(inst)
                    continue
                waits = list(si.on_wait)
                prefix = _ENGINE_SEM_PREFIX.get(str(inst.engine))
                if prefix is not None and len(waits) > limit:
                    waits = [w for w in waits if not w.ant_name.startswith(prefix)]
                if str(inst.opcode) != "Drain":
                    for droppable in _DROPPABLE.get(str(inst.engine), ()):
                        if len(waits) <= limit:
                            break
                        waits = [
                            w for w in waits if not w.ant_name.startswith(droppable)
                        ]
                keep = waits[-limit:] if limit > 0 else []
                excess = waits[: len(waits) - len(keep)]
                for w in excess:
                    carrier_id[0] += 1
                    nop = mybir.InstNoOp(name=f"WC-{carrier_id[0]}", ins=[], outs=[])
                    nop.engine = inst.engine
                    nop.sync_info = mybir.SyncInfo(on_wait=[w], on_update=[])
                    out.append(nop)
                    changed = True
                si.on_wait = keep
                out.append(inst)
                changed = True
            if changed:
                blk.instructions = out


def _get_runner(reps: int = 1):
    """Build once; return a callable(in_maps) -> list of per-core out dicts.

    Replicates bass2jax.run_bass_via_pjrt's multi-core path but caches the
    jitted shard_map executable so repeated kernel() calls skip retracing
    and recompilation.
    """
    key = ("run", reps)
    if key in _STATE:
        return _STATE[key]

    import jax
    import concourse.mybir as mybir
    from concourse import bass2jax
    from jax.experimental.shard_map import shard_map
    from jax.sharding import Mesh, PartitionSpec

    nc = _build(reps=reps)
    bass2jax.install_neuronx_cc_hook()

    partition_name = nc.partition_id_tensor.name if nc.partition_id_tensor else None

    in_names: list[str] = []
    out_names: list[str] = []
    out_avals: list = []
    for alloc in nc.m.functions[0].allocations:
        if not isinstance(alloc, mybir.MemoryLocationSet):
            continue
        name = alloc.memorylocations[0].name
        if alloc.kind == "ExternalInput":
            if name != partition_name:
                in_names.append(name)
        elif alloc.kind == "ExternalOutput":
            out_names.append(name)
            out_avals.append(
                jax.core.ShapedArray(tuple(alloc.tensor_shape), mybir.dt.np(alloc.dtype))
            )
    n_params = len(in_names)
    n_outs = len(out_avals)
    all_names = list(in_names) + out_names
    if partition_name is not None:
        all_names.append(partition_name)

    def _body(*args):
        operands = list(args)
        if partition_name is not None:
            operands.append(bass2jax.partition_id_tensor())
        outs = bass2jax._bass_exec_p.bind(
            *operands,
            out_avals=tuple(out_avals),
            in_names=tuple(all_names),
            out_names=tuple(out_names),
            lowering_input_output_aliases=(),
            sim_require_finite=True,
            sim_require_nnan=True,
            nc=nc,
        )
        return tuple(outs)

    devices = jax.devices()[:NCORES]
    mesh = Mesh(np.asarray(devices), ("core",))
    in_specs = (PartitionSpec("core"),) * (n_params + n_outs)
    out_specs = (PartitionSpec("core"),) * n_outs
    donate = tuple(range(n_params, n_params + n_outs))
    sharded = jax.jit(
        shard_map(_body, mesh=mesh, in_specs=in_specs, out_specs=out_specs, check_rep=False),
        donate_argnums=donate,
        keep_unused=True,
    )

    def run(in_maps):
        concat_in = [
            np.concatenate([np.asarray(m[name]) for m in in_maps], axis=0)
            for name in in_names
        ]
        concat_zeros = [
            np.zeros((NCORES * a.shape[0], *a.shape[1:]), a.dtype) for a in out_avals
        ]
        out_arrs = sharded(*concat_in, *concat_zeros)
        return [
            {
                name: np.asarray(out_arrs[i]).reshape(NCORES, *out_avals[i].shape)[c]
                for i, name in enumerate(out_names)
            }
            for c in range(NCORES)
        ]

    _STATE[("nc", reps)] = nc
    _STATE[key] = run
    return run


def _trunc11(v: np.ndarray) -> np.ndarray:
    """Truncate f32 mantissas to 11 explicit bits — exactly representable in
    fp32r (the PE truncates operands to 11 mantissa bits)."""
    u = np.ascontiguousarray(v, dtype=np.float32).view(np.uint32)
    return (u & np.uint32(0xFFFFF000)).view(np.float32)


def _split11(v64: np.ndarray) -> tuple[np.ndarray, np.ndarray]:
    """Split an f64 array into fp32r-exact hi plus f32 lo residual."""
    hi = _trunc11(v64.astype(np.float32))
    lo = (v64 - hi.astype(np.float64)).astype(np.float32)
    return hi, lo


def _make_in_maps(inputs: np.ndarray, preds: np.ndarray):
    xt = np.asarray(inputs, dtype=np.float32).transpose(0, 2, 1)  # [B, 3, N]
    yt = np.asarray(preds, dtype=np.float32).transpose(0, 2, 1)
    cst = np.full((B, 3, NPTS), -0.5, dtype=np.float32)
    xh, xl = _split11(xt.astype(np.float64))
    yh, yl = _split11(yt.astype(np.float64))
    x2h, x2l = _split11(xt.astype(np.float64) ** 2)
    y2h, y2l = _split11(yt.astype(np.float64) ** 2)
    # K=21 stacks; with C = -0.5 (fp32r-exact) the contraction yields
    #   sum = -x^2/2 - y^2/2 + (xh*yh + xh*yl + xl*yh)
    # where every hi row is fp32r-exact and each lo row only loses
    # O(2^-22) to the PE's truncation — near-f32 d2 despite fp32r.
    xin = np.ascontiguousarray(
        np.concatenate([x2h, x2l, xh, xh, xl, cst, cst], axis=1)
    )
    yin = np.ascontiguousarray(
        np.concatenate([cst, cst, yh, yl, yh, y2h, y2l], axis=1)
    )
    return [
        {"xin": xin[BPC * c : BPC * (c + 1)], "yin": yin[BPC * c : BPC * (c + 1)]}
        for c in range(NCORES)
    ]


def _combine(results) -> np.ndarray:
    row_total = -2.0 * sum(float(r["out"][0, 0]) for r in results)
    col_total = -2.0 * sum(float(r["out"][0, 1]) for r in results)
    mean1 = row_total / (B * NPTS)
    mean2 = col_total / (B * NPTS)
    return np.array(max(mean1, mean2), dtype=np.float32)


def kernel(inputs: np.ndarray, preds: np.ndarray) -> np.ndarray:
    run = _get_runner()
    results = run(_make_in_maps(inputs, preds))
    return _combine(results)
